# revision 24
# baseline (speedup 1.0000x reference)
"""DPLR-SSM layer kernel for Trainium2 (8 NeuronCores, batch-parallel).

Math: the reference recurrence is
    x_t = M x_{t-1} + B_bar u_t,   M = diag(A_bar) + dt * P Q^H   (n=64 complex)
    y_t = Re(C x_t) + D * u_t
M is time-invariant, so we eigendecompose M = V diag(w) V^{-1} on the host
(tiny, n=64) and run the diagonal system
    x'_t = w x'_{t-1} + B_eff u_t,  y_t = Re(C_eff x'_t) + D u_t
with B_eff = V^{-1} B_bar, C_eff = C V.  The complex diagonal scan is made
real by the phase-rotation trick: with w = rho * e^{i*theta},
z_t = e^{-i*theta*t} x'_t obeys  z_t = rho * z_{t-1} + e^{-i*theta*t} b_t,
which is two independent REAL first-order scans (hardware tensor_tensor_scan).

Per-core layout (2 batches of the 16), everything keyed on 128 partitions:
  - u is DMA'd with an fp32->bf16 casting SWDGE transfer (gpsimd ring),
    PE-transposed in bf16 (1 cyc/row) and kept as uT (d-major) for both the
    B-projection AND the D*u term.
  - rotation tables are generated ON-CHIP: one fp32 phase tensor
    ph[n,t] = theta_n * t mod 2pi (centered) is DMA'd per time-quarter and
    the 4 needed tables ([c;s], [-s;c], [c;c], [-s;s]) are each ONE scalar
    engine Sin activation with a per-partition bias in {0, pi/2, -pi}.
  - D*u enters through the C-projection PSUM accumulation as 4 diagonal
    matmuls (lhsT = uT chunk, rhs = diag(D) block) -- no elementwise D*u
    pass, no PSUM+SBUF merge pass.
  - pass pipeline: P3 (C-projection) of pass q is emitted AFTER P1/P2 of
    pass q+1 so the tensor engine never stalls on the scan chain.
"""

import math

import numpy as np

N = 64
D = 512
BATCH = 16
SEQ = 4096
NCORES = 8
BPC = BATCH // NCORES  # batches per core = 2

_PROG_CACHE = {}

# Set by test harnesses to capture a hardware profile; harmless defaults.
TRACE = False
LAST_RESULTS = None


def _host_precompute(log_neg_real, imag, P_real, P_imag, Q_real, Q_imag,
                     B_real, B_imag, C_real, C_imag, log_dt, D_vec, L):
    """All small-parameter math in float64 on host; returns device arrays."""
    import ml_dtypes

    dt = math.exp(float(np.asarray(log_dt).reshape(-1)[0]))
    Lam = -np.exp(log_neg_real.astype(np.float64)) + 1j * imag.astype(np.float64)
    A_bar = np.exp(Lam * dt)
    B = B_real.astype(np.float64) + 1j * B_imag.astype(np.float64)
    B_bar = ((A_bar - 1.0) / (Lam + 1e-8) * dt)[:, None] * B          # (n, d)
    P = P_real.astype(np.float64) + 1j * P_imag.astype(np.float64)
    Qc = Q_real.astype(np.float64) - 1j * Q_imag.astype(np.float64)
    C = C_real.astype(np.float64) + 1j * C_imag.astype(np.float64)   # (d, n)

    M = np.diag(A_bar) + dt * (P @ Qc.T)
    w, V = np.linalg.eig(M)
    B_eff = np.linalg.solve(V, B_bar)                                 # (n, d)
    C_eff = C @ V                                                     # (d, n)

    rho = np.abs(w)
    theta = np.angle(w)

    # rotation tables (bf16): tc = [cos; cos], ts = [-sin; sin]
    # rotate:  tA = tc*binb = [c*br; c*bi],  tB = ts*binb2 = [-s*bi; s*br]
    #          rot = tA - tB = e^{-i th t} (br + i bi)
    # inverse: g1 = [tc_top*zr ; ts_bot*zi] = [c*zr ;  s*zi]
    #          g2 = [ts_top*zr ; tc_bot*zi] = [-s*zr ; c*zi]
    import ml_dtypes as _mld
    t_idx = np.arange(1, L + 1, dtype=np.float64)
    ang = np.outer(theta, t_idx)                                      # (n, L)
    cos_t = np.cos(ang)
    sin_t = np.sin(ang)
    tcos = np.concatenate([cos_t, cos_t], axis=0).astype(_mld.bfloat16)
    tsin = np.concatenate([-sin_t, sin_t], axis=0).astype(_mld.bfloat16)

    # rho column (128, 1): per-partition scan coefficient
    rhoc = np.concatenate([rho, rho]).astype(np.float32).reshape(128, 1)

    # B weights, lhsT layout: bcomb[p, c*128+m] = Bc[c*128+p, m]
    # where Bc[d, m] with m=comp*64+n: comp0 -> Re(B_eff)[n,d], comp1 -> Im
    Bc = np.concatenate([B_eff.real, B_eff.imag], axis=0).T           # (512, 128)
    bcomb = Bc.reshape(4, 128, 128).transpose(1, 0, 2).reshape(128, 512)
    bcomb = np.ascontiguousarray(bcomb).astype(ml_dtypes.bfloat16)
    # component-swap permutation: binb2 = pswap^T @ binb swaps the top and
    # bottom 64 partitions ([br;bi] -> [bi;br]) in one cheap PE matmul.
    psw = np.zeros((128, 128), dtype=np.float64)
    for k in range(128):
        psw[k, (k + 64) % 128] = 1.0
    pswap = psw.astype(ml_dtypes.bfloat16)

    # C-proj weights (K on partitions): W1 rows n: Cr[d,n]; rows 64+n: -Cr[d,n]
    #                                   W2 rows n: -Ci[d,n]; rows 64+n: -Ci[d,n]
    # G1 = t1*z = [c*zr ; s*zi];  G2 = t2*z = [-s*zr ; c*zi]
    # y = sum_n Cr*(c*zr) + (-Cr)*(s*zi) + Ci*(-s*zr) + (-Ci)*(c*zi)
    Cr = C_eff.real.T                                                 # (n, d)
    Ci = C_eff.imag.T
    W1 = np.concatenate([Cr, -Cr], axis=0)                            # (128, 512)
    W2 = np.concatenate([Ci, -Ci], axis=0)
    cexp = np.concatenate([W1, W2], axis=1).astype(ml_dtypes.bfloat16)

    # diag(D) blocks for the D*u matmul: diagd[p, c*128+j] = D[c*128+p]*(p==j)
    dd = np.zeros((128, 512), dtype=np.float64)
    for c in range(4):
        np.fill_diagonal(dd[:, c * 128:(c + 1) * 128],
                         D_vec.astype(np.float64)[c * 128:(c + 1) * 128])
    diagd = dd.astype(ml_dtypes.bfloat16)

    return dict(tcos=tcos, tsin=tsin, rhoc=rhoc, bcomb=bcomb, pswap=pswap,
                cexp=cexp, diagd=diagd)


def _split_multi_waits(nc, mybir):
    """Walrus codegen only honors a single sync-wait slot on compute
    instruction structs (ACT/TS/TT...).  Move surplus waits onto chained
    EventSemaphore instructions on the same engine right before the op —
    in-order engine execution makes this equivalent."""
    n = 0
    for func in nc.m.functions:
        for blk in func.blocks:
            il = blk.instructions
            i = 0
            while i < len(il):
                inst = il[i]
                si = inst.sync_info
                if (si is not None and si.on_wait and len(si.on_wait) > 1
                        and not isinstance(inst, mybir.InstEventSemaphore)):
                    waits = list(si.on_wait)
                    for w in waits[:-1]:
                        ev = mybir.InstEventSemaphore(
                            name=f"EVW-{n}", ins=[], outs=[])
                        n += 1
                        ev.engine = inst.engine
                        ev.sync_info = mybir.SyncInfo(on_wait=[w],
                                                      on_update=[])
                        il.insert(i, ev)
                        i += 1
                    inst.sync_info = mybir.SyncInfo(on_wait=[waits[-1]],
                                                    on_update=si.on_update)
                i += 1
    return n


def _build_program(L, split_waits=True):
    """SPMD Bass program for one core: u (BPC*L, 512) -> y, processed as
    Q=4 time-quarter passes with the C-projection deferred one pass."""
    import concourse.bass as bass
    import concourse.mybir as mybir
    import concourse.tile as tile
    from concourse.masks import make_identity

    TROWS = BPC * L            # 8192 time-rows per core
    # uniform passes won on hardware: finer final passes cost more in SWDGE
    # per-transfer overhead than they save in scan->C tail.
    SIZES = [1024, 1024, 1024, 1024]
    assert sum(SIZES) == L
    Q = len(SIZES)
    OFFS = [sum(SIZES[:i]) for i in range(Q)]
    FP32 = mybir.dt.float32
    BF16 = mybir.dt.bfloat16
    Alu = mybir.AluOpType

    nc = bass.Bass()
    u_d = nc.dram_tensor("u", [TROWS, D], FP32, kind="ExternalInput")
    tcos_d = nc.dram_tensor("tcos", [128, L], BF16, kind="ExternalInput")
    tsin_d = nc.dram_tensor("tsin", [128, L], BF16, kind="ExternalInput")
    rhoc_d = nc.dram_tensor("rhoc", [128, 1], FP32, kind="ExternalInput")
    bcomb_d = nc.dram_tensor("bcomb", [128, 512], BF16, kind="ExternalInput")
    pswap_d = nc.dram_tensor("pswap", [128, 128], BF16, kind="ExternalInput")
    cexp_d = nc.dram_tensor("cexp", [128, 1024], BF16, kind="ExternalInput")
    diagd_d = nc.dram_tensor("diagd", [128, 512], BF16, kind="ExternalInput")
    y_d = nc.dram_tensor("y", [TROWS, D], FP32, kind="ExternalOutput")

    with tile.TileContext(nc) as tc:
        with (
            tc.tile_pool(name="persist", bufs=1) as pp,
            tc.tile_pool(name="ptab", bufs=2) as ptab,
            tc.tile_pool(name="pu", bufs=2) as pu,
            tc.tile_pool(name="put", bufs=2) as put,
            tc.tile_pool(name="pbin", bufs=2) as pbin,
            tc.tile_pool(name="ptmp", bufs=2) as ptmp,
            tc.tile_pool(name="pg", bufs=2) as pg,
            tc.tile_pool(name="py3", bufs=3) as py3,
            tc.tile_pool(name="psT", bufs=2, space="PSUM") as psT,
            tc.tile_pool(name="psB", bufs=1, space="PSUM") as psB,
            tc.tile_pool(name="psC", bufs=2, space="PSUM") as psC,
        ):
            bcomb_s = pp.tile([128, 512], BF16, tag="bcomb")
            pswap_s = pp.tile([128, 128], BF16, tag="pswap")
            cexp_s = pp.tile([128, 1024], BF16, tag="cexp")
            diagd_s = pp.tile([128, 512], BF16, tag="diagd")
            rhoc_s = pp.tile([128, 1], FP32, tag="rhoc")
            ident = pp.tile([128, 128], BF16, tag="ident")
            zprev = pp.tile([128, BPC], FP32, tag="zprev")
            # params ride the sync HWDGE ring so the gpsimd SWDGE ring's
            # first work is the pass-0 u slabs (startup critical path).
            nc.sync.dma_start(out=bcomb_s, in_=bcomb_d[:, :])
            nc.sync.dma_start(out=pswap_s, in_=pswap_d[:, :])
            nc.sync.dma_start(out=cexp_s, in_=cexp_d[:, :])
            nc.sync.dma_start(out=diagd_s, in_=diagd_d[:, :])
            nc.sync.dma_start(out=rhoc_s, in_=rhoc_d[:, :])

            # per-pass state carried to the deferred P3
            state = [None] * Q
            ident_made = [False]

            def emit_p1_loads(q):
                """u slab DMAs (gpsimd/SWDGE, fp32->bf16 cast) + tables."""
                TQ = SIZES[q]
                NJ = TQ // 128
                u_tiles = []
                for b in range(BPC):
                    row0 = b * L + OFFS[q]
                    u_nat = pu.tile([128, NJ * 512], BF16,
                                    tag=f"u_nat{b}_{TQ}")
                    u_tiles.append(u_nat)
                    for hh in range(2):
                        rows = slice(row0 + hh * (TQ // 2),
                                     row0 + (hh + 1) * (TQ // 2))
                        srch = u_d[rows, :].rearrange("(j p) d -> p j d",
                                                      p=128)
                        seg = u_nat[:, hh * (NJ // 2) * 512:
                                    (hh + 1) * (NJ // 2) * 512]
                        nc.gpsimd.dma_start(
                            out=seg.rearrange("p (j d) -> p j d", j=NJ // 2),
                            in_=srch)
                if not ident_made[0]:
                    # emitted after the first u loads are queued on gpsimd
                    make_identity(nc, ident)
                    nc.gpsimd.memset(zprev, 0.0)
                    ident_made[0] = True
                cs = slice(OFFS[q], OFFS[q] + TQ)
                tcs = ptab.tile([128, TQ], BF16, tag=f"tcs{TQ}")
                tss = ptab.tile([128, TQ], BF16, tag=f"tss{TQ}")
                nc.sync.dma_start(out=tcs, in_=tcos_d[:, cs])
                nc.sync.dma_start(out=tss, in_=tsin_d[:, cs])
                return u_tiles, tcs, tss

            def make_p1_units(q, u_tiles):
                """Per-(b,g) transpose + uT evac + B-projection emitters."""
                TQ = SIZES[q]
                NG = TQ // 512                    # 512-t groups per batch
                uT_tiles = [None] * (BPC * NG)
                binb = pbin.tile([128, BPC * TQ], BF16, tag=f"binb{TQ}")
                binb2 = pbin.tile([128, BPC * TQ], BF16, tag=f"binb2{TQ}")

                def unit(b, g):
                    u_nat = u_tiles[b]
                    uT = put.tile([128, 2048], BF16, tag=f"uT{b}{g}")
                    uT_tiles[b * NG + g] = uT
                    for half in range(2):         # c-chunk pairs
                        pt = psT.tile([128, 1024], BF16, tag="pt")
                        for cc in range(2):
                            c = half * 2 + cc
                            for j2 in range(4):
                                col = (g * 4 + j2) * 512 + c * 128
                                nc.tensor.transpose(
                                    pt[:, cc * 512 + j2 * 128:
                                       cc * 512 + (j2 + 1) * 128],
                                    u_nat[:, col:col + 128], ident)
                        dst = uT[:, half * 1024:(half + 1) * 1024]
                        if half == 0:
                            nc.vector.tensor_copy(dst, pt)
                        else:
                            nc.scalar.copy(dst, pt)
                    pb = psB.tile([128, 512], FP32, tag="pb")
                    for c in range(4):
                        nc.tensor.matmul(
                            pb, bcomb_s[:, c * 128:(c + 1) * 128],
                            uT[:, c * 512:(c + 1) * 512],
                            start=(c == 0), stop=(c == 3))
                    off = b * TQ + g * 512
                    nc.scalar.copy(binb[:, off:off + 512], pb)

                def swap(b, g):
                    # binb2 = pswap^T @ binb: [br;bi] -> [bi;br]
                    off = b * TQ + g * 512
                    pb2 = psB.tile([128, 512], FP32, tag="pb2")
                    nc.tensor.matmul(pb2, pswap_s, binb[:, off:off + 512],
                                     start=True, stop=True)
                    nc.scalar.copy(binb2[:, off:off + 512], pb2)

                units = [(lambda b=b, g=g: unit(b, g))
                         for b in range(BPC) for g in range(NG)]
                swaps = [(lambda b=b, g=g: swap(b, g))
                         for b in range(BPC) for g in range(NG)]
                return binb, binb2, uT_tiles, units, swaps

            def emit_p2(q, binb, binb2, uT_tiles, tcs, tss):
                TQ = SIZES[q]
                # ---------------- P2: rotate + scan + inverse rotate -------
                rhob = rhoc_s.broadcast_to([128, TQ])
                g1 = pg.tile([128, BPC * TQ], BF16, tag=f"g1_{TQ}")
                g2 = pg.tile([128, BPC * TQ], BF16, tag=f"g2_{TQ}")
                sls = [slice(b * TQ, (b + 1) * TQ) for b in range(BPC)]
                tA = []
                for b in range(BPC):
                    tmpA = ptmp.tile([128, TQ], BF16, tag=f"tmpA{b}_{TQ}")
                    tmpB = ptmp.tile([128, TQ], BF16, tag=f"tmpB{b}_{TQ}")
                    tA.append(tmpA)
                    nc.vector.tensor_mul(tmpA, tcs, binb[:, sls[b]])
                    nc.vector.tensor_mul(tmpB, tss, binb2[:, sls[b]])
                    nc.vector.tensor_sub(binb[:, sls[b]], tmpA, tmpB)
                for b in range(BPC):
                    nc.vector.tensor_tensor_scan(
                        tA[b], rhob, binb[:, sls[b]],
                        zprev[:, b:b + 1], Alu.mult, Alu.add)
                    nc.vector.tensor_copy(zprev[:, b:b + 1],
                                          tA[b][:, TQ - 1:TQ])
                for b in range(BPC):
                    zt = tA[b]
                    g1b = g1[:, sls[b]]
                    g2b = g2[:, sls[b]]
                    nc.vector.tensor_mul(g1b[0:64, :], tcs[0:64, :],
                                         zt[0:64, :])
                    nc.vector.tensor_mul(g1b[64:128, :], tss[64:128, :],
                                         zt[64:128, :])
                    nc.vector.tensor_mul(g2b[0:64, :], tss[0:64, :],
                                         zt[0:64, :])
                    nc.vector.tensor_mul(g2b[64:128, :], tcs[64:128, :],
                                         zt[64:128, :])

                state[q] = (g1, g2, uT_tiles)

            def p3_pair(q, b, pair):
                # ---------------- P3: C-projection + D*u + store -----------
                TQ = SIZES[q]
                NG = TQ // 512
                g1, g2, uT_tiles = state[q]
                row0 = b * L + OFFS[q]
                py = psC.tile([128, 1024], FP32, tag="py")
                for sub in range(2):
                    jj = pair * 2 + sub
                    g = jj // 4
                    j2 = jj % 4
                    off = b * TQ + jj * 128
                    uT = uT_tiles[b * NG + g]
                    ps_ = py[:, sub * 512:(sub + 1) * 512]
                    # full-region matmuls FIRST: start=True zeroes the whole
                    # region, so partial-region (diag) accumulate after.
                    nc.tensor.matmul(ps_, g1[:, off:off + 128],
                                     cexp_s[:, 0:512],
                                     start=True, stop=False,
                                     skip_group_check=True)
                    nc.tensor.matmul(ps_, g2[:, off:off + 128],
                                     cexp_s[:, 512:1024],
                                     start=False, stop=False,
                                     skip_group_check=True)
                    for c in range(4):            # D*u diagonal blocks
                        nc.tensor.matmul(
                            ps_[:, c * 128:(c + 1) * 128],
                            uT[:, c * 512 + j2 * 128:
                               c * 512 + (j2 + 1) * 128],
                            diagd_s[:, c * 128:(c + 1) * 128],
                            start=False, stop=(c == 3),
                            skip_group_check=True)
                ysl = py3.tile([128, 1024], FP32, tag="ysl")
                nc.scalar.copy(ysl, py)
                rows = slice(row0 + pair * 256, row0 + (pair + 1) * 256)
                dst = y_d[rows, :].rearrange("(j p) d -> p j d", p=128)
                nc.sync.dma_start(out=dst,
                                  in_=ysl.rearrange("p (j d) -> p j d", j=2))

            def p3_pairs(q):
                TQ = SIZES[q]
                return [(lambda b=b, pair=pair: p3_pair(q, b, pair))
                        for b in range(BPC) for pair in range(TQ // 256)]

            # pass q's P1 units run first (swap matmuls woven one unit
            # behind so the PE never waits on the binb evacuation), then
            # pass q-1's deferred C-projection, then pass q's scan.
            for q in range(Q):
                u_tiles, tcs, tss = emit_p1_loads(q)
                binb, binb2, uT_tiles, units, swaps = \
                    make_p1_units(q, u_tiles)
                for i, fn in enumerate(units):
                    fn()
                    if i >= 1:
                        swaps[i - 1]()
                swaps[-1]()
                for fn in (p3_pairs(q - 1) if q > 0 else []):
                    fn()
                emit_p2(q, binb, binb2, uT_tiles, tcs, tss)
            for fn in p3_pairs(Q - 1):
                fn()

    if split_waits:
        _split_multi_waits(nc, mybir)
    return nc


def kernel(**inputs):
    from concourse.bass_utils import run_bass_kernel_spmd

    u = np.ascontiguousarray(inputs["u"], dtype=np.float32)
    L = u.shape[1]
    params = _host_precompute(
        inputs["log_neg_real"], inputs["imag"], inputs["P_real"],
        inputs["P_imag"], inputs["Q_real"], inputs["Q_imag"],
        inputs["B_real"], inputs["B_imag"], inputs["C_real"],
        inputs["C_imag"], inputs["log_dt"], inputs["D"], L)

    if L not in _PROG_CACHE:
        _PROG_CACHE[L] = _build_program(L)
    nc = _PROG_CACHE[L]

    in_maps = []
    for c in range(NCORES):
        shard = np.ascontiguousarray(
            u[c * BPC:(c + 1) * BPC].reshape(BPC * L, u.shape[2]))
        m = {"u": shard}
        m.update(params)
        in_maps.append(m)

    kwargs = {}
    if TRACE:
        kwargs = dict(trace=True, stitch_traces=False)
    res = run_bass_kernel_spmd(nc, in_maps, core_ids=list(range(NCORES)),
                               **kwargs)
    global LAST_RESULTS
    LAST_RESULTS = res
    y = np.empty_like(u)
    for c in range(NCORES):
        y[c * BPC:(c + 1) * BPC] = res.results[c]["y"].reshape(BPC, L, u.shape[2])
    return y


# revision 31
# speedup vs baseline: 1.0120x; 1.0120x over previous
"""DPLR-SSM layer kernel for Trainium2 (8 NeuronCores, batch-parallel).

Math: the reference recurrence is
    x_t = M x_{t-1} + B_bar u_t,   M = diag(A_bar) + dt * P Q^H   (n=64 complex)
    y_t = Re(C x_t) + D * u_t
M is time-invariant, so we eigendecompose M = V diag(w) V^{-1} on the host
(tiny, n=64) and run the diagonal system
    x'_t = w x'_{t-1} + B_eff u_t,  y_t = Re(C_eff x'_t) + D u_t
with B_eff = V^{-1} B_bar, C_eff = C V.  The complex diagonal scan is made
real by the phase-rotation trick: with w = rho * e^{i*theta},
z_t = e^{-i*theta*t} x'_t obeys  z_t = rho * z_{t-1} + e^{-i*theta*t} b_t,
which is two independent REAL first-order scans (hardware tensor_tensor_scan).

Per-core layout (2 batches of the 16), everything keyed on 128 partitions:
  - u is DMA'd with an fp32->bf16 casting SWDGE transfer (gpsimd ring),
    PE-transposed in bf16 (1 cyc/row) and kept as uT (d-major) for both the
    B-projection AND the D*u term.
  - rotation tables are generated ON-CHIP: one fp32 phase tensor
    ph[n,t] = theta_n * t mod 2pi (centered) is DMA'd per time-quarter and
    the 4 needed tables ([c;s], [-s;c], [c;c], [-s;s]) are each ONE scalar
    engine Sin activation with a per-partition bias in {0, pi/2, -pi}.
  - D*u enters through the C-projection PSUM accumulation as 4 diagonal
    matmuls (lhsT = uT chunk, rhs = diag(D) block) -- no elementwise D*u
    pass, no PSUM+SBUF merge pass.
  - pass pipeline: P3 (C-projection) of pass q is emitted AFTER P1/P2 of
    pass q+1 so the tensor engine never stalls on the scan chain.
"""

import math

import numpy as np

N = 64
D = 512
BATCH = 16
SEQ = 4096
NCORES = 8
BPC = BATCH // NCORES  # batches per core = 2

_PROG_CACHE = {}

# Set by test harnesses to capture a hardware profile; harmless defaults.
TRACE = False
LAST_RESULTS = None


def _host_precompute(log_neg_real, imag, P_real, P_imag, Q_real, Q_imag,
                     B_real, B_imag, C_real, C_imag, log_dt, D_vec, L):
    """All small-parameter math in float64 on host; returns device arrays."""
    import ml_dtypes

    dt = math.exp(float(np.asarray(log_dt).reshape(-1)[0]))
    Lam = -np.exp(log_neg_real.astype(np.float64)) + 1j * imag.astype(np.float64)
    A_bar = np.exp(Lam * dt)
    B = B_real.astype(np.float64) + 1j * B_imag.astype(np.float64)
    B_bar = ((A_bar - 1.0) / (Lam + 1e-8) * dt)[:, None] * B          # (n, d)
    P = P_real.astype(np.float64) + 1j * P_imag.astype(np.float64)
    Qc = Q_real.astype(np.float64) - 1j * Q_imag.astype(np.float64)
    C = C_real.astype(np.float64) + 1j * C_imag.astype(np.float64)   # (d, n)

    M = np.diag(A_bar) + dt * (P @ Qc.T)
    w, V = np.linalg.eig(M)
    B_eff = np.linalg.solve(V, B_bar)                                 # (n, d)
    C_eff = C @ V                                                     # (d, n)

    rho = np.abs(w)
    theta = np.angle(w)

    # rotation tables (bf16): tc = [cos; cos], ts = [-sin; sin]
    # rotate:  tA = tc*binb = [c*br; c*bi],  tB = ts*binb2 = [-s*bi; s*br]
    #          rot = tA - tB = e^{-i th t} (br + i bi)
    # inverse: g1 = [tc_top*zr ; ts_bot*zi] = [c*zr ;  s*zi]
    #          g2 = [ts_top*zr ; tc_bot*zi] = [-s*zr ; c*zi]
    import ml_dtypes as _mld
    t_idx = np.arange(1, L + 1, dtype=np.float64)
    ang = np.outer(theta, t_idx)                                      # (n, L)
    cos_t = np.cos(ang)
    sin_t = np.sin(ang)
    tcos = np.concatenate([cos_t, cos_t], axis=0).astype(_mld.bfloat16)
    tsin = np.concatenate([-sin_t, sin_t], axis=0).astype(_mld.bfloat16)

    # rho column (128, 1): per-partition scan coefficient
    rhoc = np.concatenate([rho, rho]).astype(np.float32).reshape(128, 1)

    # B weights, lhsT layout: bcomb[p, c*128+m] = Bc[c*128+p, m]
    # where Bc[d, m] with m=comp*64+n: comp0 -> Re(B_eff)[n,d], comp1 -> Im
    Bc = np.concatenate([B_eff.real, B_eff.imag], axis=0).T           # (512, 128)
    bcomb = Bc.reshape(4, 128, 128).transpose(1, 0, 2).reshape(128, 512)
    bcomb = np.ascontiguousarray(bcomb).astype(ml_dtypes.bfloat16)
    # component-swapped variant: bs2 = [bi ; br] comes straight from PE
    Bc2 = np.concatenate([B_eff.imag, B_eff.real], axis=0).T          # (512, 128)
    bcomb2 = Bc2.reshape(4, 128, 128).transpose(1, 0, 2).reshape(128, 512)
    bcomb2 = np.ascontiguousarray(bcomb2).astype(ml_dtypes.bfloat16)

    # C-proj weights (K on partitions): W1 rows n: Cr[d,n]; rows 64+n: -Cr[d,n]
    #                                   W2 rows n: -Ci[d,n]; rows 64+n: -Ci[d,n]
    # G1 = t1*z = [c*zr ; s*zi];  G2 = t2*z = [-s*zr ; c*zi]
    # y = sum_n Cr*(c*zr) + (-Cr)*(s*zi) + Ci*(-s*zr) + (-Ci)*(c*zi)
    Cr = C_eff.real.T                                                 # (n, d)
    Ci = C_eff.imag.T
    W1 = np.concatenate([Cr, -Cr], axis=0)                            # (128, 512)
    W2 = np.concatenate([Ci, -Ci], axis=0)
    cexp = np.concatenate([W1, W2], axis=1).astype(ml_dtypes.bfloat16)

    # diag(D) blocks for the D*u matmul: diagd[p, c*128+j] = D[c*128+p]*(p==j)
    dd = np.zeros((128, 512), dtype=np.float64)
    for c in range(4):
        np.fill_diagonal(dd[:, c * 128:(c + 1) * 128],
                         D_vec.astype(np.float64)[c * 128:(c + 1) * 128])
    diagd = dd.astype(ml_dtypes.bfloat16)

    return dict(tcos=tcos, tsin=tsin, rhoc=rhoc, bcomb=bcomb, bcomb2=bcomb2,
                cexp=cexp, diagd=diagd)


def _split_multi_waits(nc, mybir):
    """Walrus codegen only honors a single sync-wait slot on compute
    instruction structs (ACT/TS/TT...).  Move surplus waits onto chained
    EventSemaphore instructions on the same engine right before the op —
    in-order engine execution makes this equivalent."""
    n = 0
    for func in nc.m.functions:
        for blk in func.blocks:
            il = blk.instructions
            i = 0
            while i < len(il):
                inst = il[i]
                si = inst.sync_info
                if (si is not None and si.on_wait and len(si.on_wait) > 1
                        and not isinstance(inst, mybir.InstEventSemaphore)):
                    waits = list(si.on_wait)
                    for w in waits[:-1]:
                        ev = mybir.InstEventSemaphore(
                            name=f"EVW-{n}", ins=[], outs=[])
                        n += 1
                        ev.engine = inst.engine
                        ev.sync_info = mybir.SyncInfo(on_wait=[w],
                                                      on_update=[])
                        il.insert(i, ev)
                        i += 1
                    inst.sync_info = mybir.SyncInfo(on_wait=[waits[-1]],
                                                    on_update=si.on_update)
                i += 1
    return n


def _build_program(L, split_waits=True):
    """SPMD Bass program for one core: u (BPC*L, 512) -> y, processed as
    Q=4 time-quarter passes with the C-projection deferred one pass."""
    import concourse.bass as bass
    import concourse.mybir as mybir
    import concourse.tile as tile
    from concourse.masks import make_identity

    TROWS = BPC * L            # 8192 time-rows per core
    # uniform passes won on hardware: finer final passes cost more in SWDGE
    # per-transfer overhead than they save in scan->C tail.
    SIZES = [1024, 1024, 1024, 1024]
    assert sum(SIZES) == L
    Q = len(SIZES)
    OFFS = [sum(SIZES[:i]) for i in range(Q)]
    FP32 = mybir.dt.float32
    BF16 = mybir.dt.bfloat16
    Alu = mybir.AluOpType

    nc = bass.Bass()
    u_d = nc.dram_tensor("u", [TROWS, D], FP32, kind="ExternalInput")
    tcos_d = nc.dram_tensor("tcos", [128, L], BF16, kind="ExternalInput")
    tsin_d = nc.dram_tensor("tsin", [128, L], BF16, kind="ExternalInput")
    rhoc_d = nc.dram_tensor("rhoc", [128, 1], FP32, kind="ExternalInput")
    bcomb_d = nc.dram_tensor("bcomb", [128, 512], BF16, kind="ExternalInput")
    bcomb2_d = nc.dram_tensor("bcomb2", [128, 512], BF16, kind="ExternalInput")
    cexp_d = nc.dram_tensor("cexp", [128, 1024], BF16, kind="ExternalInput")
    diagd_d = nc.dram_tensor("diagd", [128, 512], BF16, kind="ExternalInput")
    y_d = nc.dram_tensor("y", [TROWS, D], FP32, kind="ExternalOutput")

    with tile.TileContext(nc) as tc:
        with (
            tc.tile_pool(name="persist", bufs=1) as pp,
            tc.tile_pool(name="ptab", bufs=2) as ptab,
            tc.tile_pool(name="pu", bufs=2) as pu,
            tc.tile_pool(name="put", bufs=2) as put,
            tc.tile_pool(name="pbin", bufs=2) as pbin,
            tc.tile_pool(name="ptmp", bufs=2) as ptmp,
            tc.tile_pool(name="pg", bufs=2) as pg,
            tc.tile_pool(name="py3", bufs=3) as py3,
            tc.tile_pool(name="psT", bufs=2, space="PSUM") as psT,
            tc.tile_pool(name="psB", bufs=1, space="PSUM") as psB,
            tc.tile_pool(name="psC", bufs=2, space="PSUM") as psC,
        ):
            bcomb_s = pp.tile([128, 512], BF16, tag="bcomb")
            bcomb2_s = pp.tile([128, 512], BF16, tag="bcomb2")
            cexp_s = pp.tile([128, 1024], BF16, tag="cexp")
            diagd_s = pp.tile([128, 512], BF16, tag="diagd")
            rhoc_s = pp.tile([128, 1], FP32, tag="rhoc")
            ident = pp.tile([128, 128], BF16, tag="ident")
            zprev = pp.tile([128, BPC], FP32, tag="zprev")
            # params ride the sync HWDGE ring so the gpsimd SWDGE ring's
            # first work is the pass-0 u slabs (startup critical path).
            nc.sync.dma_start(out=bcomb_s, in_=bcomb_d[:, :])
            nc.sync.dma_start(out=bcomb2_s, in_=bcomb2_d[:, :])
            nc.sync.dma_start(out=cexp_s, in_=cexp_d[:, :])
            nc.sync.dma_start(out=diagd_s, in_=diagd_d[:, :])
            nc.sync.dma_start(out=rhoc_s, in_=rhoc_d[:, :])

            # per-pass state carried to the deferred P3
            state = [None] * Q
            ident_made = [False]

            def emit_p1_loads(q):
                """u slab DMAs (gpsimd/SWDGE, fp32->bf16 cast) + tables."""
                TQ = SIZES[q]
                NJ = TQ // 128
                u_tiles = []
                for b in range(BPC):
                    row0 = b * L + OFFS[q]
                    u_nat = pu.tile([128, NJ * 512], BF16,
                                    tag=f"u_nat{b}_{TQ}")
                    u_tiles.append(u_nat)
                    for hh in range(2):
                        rows = slice(row0 + hh * (TQ // 2),
                                     row0 + (hh + 1) * (TQ // 2))
                        srch = u_d[rows, :].rearrange("(j p) d -> p j d",
                                                      p=128)
                        seg = u_nat[:, hh * (NJ // 2) * 512:
                                    (hh + 1) * (NJ // 2) * 512]
                        nc.gpsimd.dma_start(
                            out=seg.rearrange("p (j d) -> p j d", j=NJ // 2),
                            in_=srch)
                if not ident_made[0]:
                    # emitted after the first u loads are queued on gpsimd
                    make_identity(nc, ident)
                    nc.gpsimd.memset(zprev, 0.0)
                    ident_made[0] = True
                cs = slice(OFFS[q], OFFS[q] + TQ)
                tcs = ptab.tile([128, TQ], BF16, tag=f"tcs{TQ}")
                tss = ptab.tile([128, TQ], BF16, tag=f"tss{TQ}")
                nc.sync.dma_start(out=tcs, in_=tcos_d[:, cs])
                nc.sync.dma_start(out=tss, in_=tsin_d[:, cs])
                return u_tiles, tcs, tss

            def make_p1_units(q, u_tiles):
                """Per-(b,g) transpose + uT evac + B-projection emitters."""
                TQ = SIZES[q]
                NG = TQ // 512                    # 512-t groups per batch
                uT_tiles = [None] * (BPC * NG)
                binb = pbin.tile([128, BPC * TQ], BF16, tag=f"binb{TQ}")
                binb2 = pbin.tile([128, BPC * TQ], BF16, tag=f"binb2{TQ}")

                def unit(b, g):
                    u_nat = u_tiles[b]
                    uT = put.tile([128, 2048], BF16, tag=f"uT{b}{g}")
                    uT_tiles[b * NG + g] = uT
                    for half in range(2):         # c-chunk pairs
                        pt = psT.tile([128, 1024], BF16, tag="pt")
                        for cc in range(2):
                            c = half * 2 + cc
                            for j2 in range(4):
                                col = (g * 4 + j2) * 512 + c * 128
                                nc.tensor.transpose(
                                    pt[:, cc * 512 + j2 * 128:
                                       cc * 512 + (j2 + 1) * 128],
                                    u_nat[:, col:col + 128], ident)
                        dst = uT[:, half * 1024:(half + 1) * 1024]
                        if half == 0:
                            nc.vector.tensor_copy(dst, pt)
                        else:
                            nc.scalar.copy(dst, pt)
                    pb = psB.tile([128, 512], FP32, tag="pb")
                    pb2 = psB.tile([128, 512], FP32, tag="pb2")
                    for c in range(4):
                        nc.tensor.matmul(
                            pb, bcomb_s[:, c * 128:(c + 1) * 128],
                            uT[:, c * 512:(c + 1) * 512],
                            start=(c == 0), stop=(c == 3))
                    for c in range(4):
                        nc.tensor.matmul(
                            pb2, bcomb2_s[:, c * 128:(c + 1) * 128],
                            uT[:, c * 512:(c + 1) * 512],
                            start=(c == 0), stop=(c == 3))
                    off = b * TQ + g * 512
                    nc.scalar.copy(binb[:, off:off + 512], pb)
                    nc.scalar.copy(binb2[:, off:off + 512], pb2)

                return binb, binb2, uT_tiles, unit

            def emit_p2_batch(q, batch, binb, binb2, tcs, tss):
                TQ = SIZES[q]
                # ---------------- P2: rotate + scan + inverse rotate -------
                # one BATCH at a time so the next pipeline stage (this
                # batch's C-projection) can start while the other batch's
                # scan chain is still running on the vector engine.
                rhob = rhoc_s.broadcast_to([128, TQ])
                g1, g2 = gtiles[q]
                b = batch
                sl = slice(b * TQ, (b + 1) * TQ)
                tmpA = ptmp.tile([128, TQ], BF16, tag=f"tmpA{b}_{TQ}")
                tmpB = ptmp.tile([128, TQ], BF16, tag=f"tmpB{b}_{TQ}")
                nc.vector.tensor_mul(tmpA, tcs, binb[:, sl])
                nc.vector.tensor_mul(tmpB, tss, binb2[:, sl])
                nc.vector.tensor_sub(binb[:, sl], tmpA, tmpB)
                nc.vector.tensor_tensor_scan(
                    tmpA, rhob, binb[:, sl],
                    zprev[:, b:b + 1], Alu.mult, Alu.add)
                nc.vector.tensor_copy(zprev[:, b:b + 1],
                                      tmpA[:, TQ - 1:TQ])
                zt = tmpA
                g1b = g1[:, sl]
                g2b = g2[:, sl]
                nc.vector.tensor_mul(g1b[0:64, :], tcs[0:64, :],
                                     zt[0:64, :])
                nc.vector.tensor_mul(g1b[64:128, :], tss[64:128, :],
                                     zt[64:128, :])
                nc.vector.tensor_mul(g2b[0:64, :], tss[0:64, :],
                                     zt[0:64, :])
                nc.vector.tensor_mul(g2b[64:128, :], tcs[64:128, :],
                                     zt[64:128, :])

            def p3_pair(q, b, pair):
                # ---------------- P3: C-projection + D*u + store -----------
                TQ = SIZES[q]
                NG = TQ // 512
                g1, g2 = gtiles[q]
                uT_tiles = state[q]
                row0 = b * L + OFFS[q]
                py = psC.tile([128, 1024], FP32, tag="py")
                for sub in range(2):
                    jj = pair * 2 + sub
                    g = jj // 4
                    j2 = jj % 4
                    off = b * TQ + jj * 128
                    uT = uT_tiles[b * NG + g]
                    ps_ = py[:, sub * 512:(sub + 1) * 512]
                    # full-region matmuls FIRST: start=True zeroes the whole
                    # region, so partial-region (diag) accumulate after.
                    nc.tensor.matmul(ps_, g1[:, off:off + 128],
                                     cexp_s[:, 0:512],
                                     start=True, stop=False,
                                     skip_group_check=True)
                    nc.tensor.matmul(ps_, g2[:, off:off + 128],
                                     cexp_s[:, 512:1024],
                                     start=False, stop=False,
                                     skip_group_check=True)
                    for c in range(4):            # D*u diagonal blocks
                        nc.tensor.matmul(
                            ps_[:, c * 128:(c + 1) * 128],
                            uT[:, c * 512 + j2 * 128:
                               c * 512 + (j2 + 1) * 128],
                            diagd_s[:, c * 128:(c + 1) * 128],
                            start=False, stop=(c == 3),
                            skip_group_check=True)
                ysl = py3.tile([128, 1024], FP32, tag="ysl")
                nc.scalar.copy(ysl, py)
                rows = slice(row0 + pair * 256, row0 + (pair + 1) * 256)
                dst = y_d[rows, :].rearrange("(j p) d -> p j d", p=128)
                nc.sync.dma_start(out=dst,
                                  in_=ysl.rearrange("p (j d) -> p j d", j=2))

            def p3_batch(q, b):
                TQ = SIZES[q]
                for pair in range(TQ // 256):
                    p3_pair(q, b, pair)

            # Fine-grained batch-stage pipeline: each stage is one batch of
            # one pass (transpose+B -> rotate+scan+g), and the previous
            # stage's C-projection is emitted right after, so the tensor
            # engine works on stage k's P1 / stage k-1's C while the vector
            # engine runs stage k's scan chain.
            gtiles = [None] * Q
            prev_stage = [None]
            for q in range(Q):
                u_tiles, tcs, tss = emit_p1_loads(q)
                binb, binb2, uT_tiles, unit = make_p1_units(q, u_tiles)
                state[q] = uT_tiles
                TQ = SIZES[q]
                g1t = pg.tile([128, BPC * TQ], BF16, tag=f"g1_{TQ}")
                g2t = pg.tile([128, BPC * TQ], BF16, tag=f"g2_{TQ}")
                gtiles[q] = (g1t, g2t)
                for b in range(BPC):
                    for g in range(TQ // 512):
                        unit(b, g)
                    emit_p2_batch(q, b, binb, binb2, tcs, tss)
                    if prev_stage[0] is not None:
                        p3_batch(*prev_stage[0])
                    prev_stage[0] = (q, b)
            p3_batch(*prev_stage[0])

    if split_waits:
        _split_multi_waits(nc, mybir)
    return nc


def kernel(**inputs):
    from concourse.bass_utils import run_bass_kernel_spmd

    u = np.ascontiguousarray(inputs["u"], dtype=np.float32)
    L = u.shape[1]
    params = _host_precompute(
        inputs["log_neg_real"], inputs["imag"], inputs["P_real"],
        inputs["P_imag"], inputs["Q_real"], inputs["Q_imag"],
        inputs["B_real"], inputs["B_imag"], inputs["C_real"],
        inputs["C_imag"], inputs["log_dt"], inputs["D"], L)

    if L not in _PROG_CACHE:
        _PROG_CACHE[L] = _build_program(L)
    nc = _PROG_CACHE[L]

    in_maps = []
    for c in range(NCORES):
        shard = np.ascontiguousarray(
            u[c * BPC:(c + 1) * BPC].reshape(BPC * L, u.shape[2]))
        m = {"u": shard}
        m.update(params)
        in_maps.append(m)

    kwargs = {}
    if TRACE:
        kwargs = dict(trace=True, stitch_traces=False)
    res = run_bass_kernel_spmd(nc, in_maps, core_ids=list(range(NCORES)),
                               **kwargs)
    global LAST_RESULTS
    LAST_RESULTS = res
    y = np.empty_like(u)
    for c in range(NCORES):
        y[c * BPC:(c + 1) * BPC] = res.results[c]["y"].reshape(BPC, L, u.shape[2])
    return y


# revision 33
# speedup vs baseline: 1.0420x; 1.0296x over previous
"""DPLR-SSM layer kernel for Trainium2 (8 NeuronCores, batch-parallel).

Math: the reference recurrence is
    x_t = M x_{t-1} + B_bar u_t,   M = diag(A_bar) + dt * P Q^H   (n=64 complex)
    y_t = Re(C x_t) + D * u_t
M is time-invariant, so we eigendecompose M = V diag(w) V^{-1} on the host
(tiny, n=64) and run the diagonal system
    x'_t = w x'_{t-1} + B_eff u_t,  y_t = Re(C_eff x'_t) + D u_t
with B_eff = V^{-1} B_bar, C_eff = C V.  The complex diagonal scan is made
real by the phase-rotation trick: with w = rho * e^{i*theta},
z_t = e^{-i*theta*t} x'_t obeys  z_t = rho * z_{t-1} + e^{-i*theta*t} b_t,
which is two independent REAL first-order scans (hardware tensor_tensor_scan).

Per-core layout (2 batches of the 16), everything keyed on 128 partitions:
  - u is DMA'd with an fp32->bf16 casting SWDGE transfer (gpsimd ring),
    PE-transposed in bf16 (1 cyc/row) and kept as uT (d-major) for both the
    B-projection AND the D*u term.
  - rotation tables are generated ON-CHIP: one fp32 phase tensor
    ph[n,t] = theta_n * t mod 2pi (centered) is DMA'd per time-quarter and
    the 4 needed tables ([c;s], [-s;c], [c;c], [-s;s]) are each ONE scalar
    engine Sin activation with a per-partition bias in {0, pi/2, -pi}.
  - D*u enters through the C-projection PSUM accumulation as 4 diagonal
    matmuls (lhsT = uT chunk, rhs = diag(D) block) -- no elementwise D*u
    pass, no PSUM+SBUF merge pass.
  - pass pipeline: P3 (C-projection) of pass q is emitted AFTER P1/P2 of
    pass q+1 so the tensor engine never stalls on the scan chain.
"""

import math

import numpy as np

N = 64
D = 512
BATCH = 16
SEQ = 4096
NCORES = 8
BPC = BATCH // NCORES  # batches per core = 2

_PROG_CACHE = {}

# Set by test harnesses to capture a hardware profile; harmless defaults.
TRACE = False
LAST_RESULTS = None


def _host_precompute(log_neg_real, imag, P_real, P_imag, Q_real, Q_imag,
                     B_real, B_imag, C_real, C_imag, log_dt, D_vec, L):
    """All small-parameter math in float64 on host; returns device arrays."""
    import ml_dtypes

    dt = math.exp(float(np.asarray(log_dt).reshape(-1)[0]))
    Lam = -np.exp(log_neg_real.astype(np.float64)) + 1j * imag.astype(np.float64)
    A_bar = np.exp(Lam * dt)
    B = B_real.astype(np.float64) + 1j * B_imag.astype(np.float64)
    B_bar = ((A_bar - 1.0) / (Lam + 1e-8) * dt)[:, None] * B          # (n, d)
    P = P_real.astype(np.float64) + 1j * P_imag.astype(np.float64)
    Qc = Q_real.astype(np.float64) - 1j * Q_imag.astype(np.float64)
    C = C_real.astype(np.float64) + 1j * C_imag.astype(np.float64)   # (d, n)

    M = np.diag(A_bar) + dt * (P @ Qc.T)
    w, V = np.linalg.eig(M)
    B_eff = np.linalg.solve(V, B_bar)                                 # (n, d)
    C_eff = C @ V                                                     # (d, n)

    rho = np.abs(w)
    theta = np.angle(w)

    # rotation tables (bf16): tc = [cos; cos], ts = [-sin; sin]
    # rotate:  tA = tc*binb = [c*br; c*bi],  tB = ts*binb2 = [-s*bi; s*br]
    #          rot = tA - tB = e^{-i th t} (br + i bi)
    # inverse: g1 = [tc_top*zr ; ts_bot*zi] = [c*zr ;  s*zi]
    #          g2 = [ts_top*zr ; tc_bot*zi] = [-s*zr ; c*zi]
    import ml_dtypes as _mld
    t_idx = np.arange(1, L + 1, dtype=np.float64)
    ang = np.outer(theta, t_idx)                                      # (n, L)
    cos_t = np.cos(ang)
    sin_t = np.sin(ang)
    tcos = np.concatenate([cos_t, cos_t], axis=0).astype(_mld.bfloat16)
    tsin = np.concatenate([-sin_t, sin_t], axis=0).astype(_mld.bfloat16)

    # rho column (128, 1): per-partition scan coefficient
    rhoc = np.concatenate([rho, rho]).astype(np.float32).reshape(128, 1)

    # B weights, lhsT layout: bcomb[p, c*128+m] = Bc[c*128+p, m]
    # where Bc[d, m] with m=comp*64+n: comp0 -> Re(B_eff)[n,d], comp1 -> Im
    Bc = np.concatenate([B_eff.real, B_eff.imag], axis=0).T           # (512, 128)
    bcomb = Bc.reshape(4, 128, 128).transpose(1, 0, 2).reshape(128, 512)
    bcomb = np.ascontiguousarray(bcomb).astype(ml_dtypes.bfloat16)
    # component-swapped variant: bs2 = [bi ; br] comes straight from PE
    Bc2 = np.concatenate([B_eff.imag, B_eff.real], axis=0).T          # (512, 128)
    bcomb2 = Bc2.reshape(4, 128, 128).transpose(1, 0, 2).reshape(128, 512)
    bcomb2 = np.ascontiguousarray(bcomb2).astype(ml_dtypes.bfloat16)

    # C-proj weights (K on partitions): W1 rows n: Cr[d,n]; rows 64+n: -Cr[d,n]
    #                                   W2 rows n: -Ci[d,n]; rows 64+n: -Ci[d,n]
    # G1 = t1*z = [c*zr ; s*zi];  G2 = t2*z = [-s*zr ; c*zi]
    # y = sum_n Cr*(c*zr) + (-Cr)*(s*zi) + Ci*(-s*zr) + (-Ci)*(c*zi)
    Cr = C_eff.real.T                                                 # (n, d)
    Ci = C_eff.imag.T
    W1 = np.concatenate([Cr, -Cr], axis=0)                            # (128, 512)
    W2 = np.concatenate([Ci, -Ci], axis=0)
    cexp = np.concatenate([W1, W2], axis=1).astype(ml_dtypes.bfloat16)

    # diag(D) blocks for the D*u matmul: diagd[p, c*128+j] = D[c*128+p]*(p==j)
    dd = np.zeros((128, 512), dtype=np.float64)
    for c in range(4):
        np.fill_diagonal(dd[:, c * 128:(c + 1) * 128],
                         D_vec.astype(np.float64)[c * 128:(c + 1) * 128])
    diagd = dd.astype(ml_dtypes.bfloat16)

    return dict(tcos=tcos, tsin=tsin, rhoc=rhoc, bcomb=bcomb, bcomb2=bcomb2,
                cexp=cexp, diagd=diagd)


def _split_multi_waits(nc, mybir):
    """Walrus codegen only honors a single sync-wait slot on compute
    instruction structs (ACT/TS/TT...).  Move surplus waits onto chained
    EventSemaphore instructions on the same engine right before the op —
    in-order engine execution makes this equivalent."""
    n = 0
    for func in nc.m.functions:
        for blk in func.blocks:
            il = blk.instructions
            i = 0
            while i < len(il):
                inst = il[i]
                si = inst.sync_info
                if (si is not None and si.on_wait and len(si.on_wait) > 1
                        and not isinstance(inst, mybir.InstEventSemaphore)):
                    waits = list(si.on_wait)
                    for w in waits[:-1]:
                        ev = mybir.InstEventSemaphore(
                            name=f"EVW-{n}", ins=[], outs=[])
                        n += 1
                        ev.engine = inst.engine
                        ev.sync_info = mybir.SyncInfo(on_wait=[w],
                                                      on_update=[])
                        il.insert(i, ev)
                        i += 1
                    inst.sync_info = mybir.SyncInfo(on_wait=[waits[-1]],
                                                    on_update=si.on_update)
                i += 1
    return n


def _build_program(L, split_waits=True):
    """SPMD Bass program for one core: u (BPC*L, 512) -> y, processed as
    Q=4 time-quarter passes with the C-projection deferred one pass."""
    import concourse.bass as bass
    import concourse.mybir as mybir
    import concourse.tile as tile
    from concourse.masks import make_identity

    TROWS = BPC * L            # 8192 time-rows per core
    # uniform passes won on hardware: finer final passes cost more in SWDGE
    # per-transfer overhead than they save in scan->C tail.
    SIZES = [1024, 1024, 1024, 1024]
    assert sum(SIZES) == L
    Q = len(SIZES)
    OFFS = [sum(SIZES[:i]) for i in range(Q)]
    FP32 = mybir.dt.float32
    BF16 = mybir.dt.bfloat16
    Alu = mybir.AluOpType

    nc = bass.Bass()
    u_d = nc.dram_tensor("u", [TROWS, D], FP32, kind="ExternalInput")
    tcos_d = nc.dram_tensor("tcos", [128, L], BF16, kind="ExternalInput")
    tsin_d = nc.dram_tensor("tsin", [128, L], BF16, kind="ExternalInput")
    rhoc_d = nc.dram_tensor("rhoc", [128, 1], FP32, kind="ExternalInput")
    bcomb_d = nc.dram_tensor("bcomb", [128, 512], BF16, kind="ExternalInput")
    bcomb2_d = nc.dram_tensor("bcomb2", [128, 512], BF16, kind="ExternalInput")
    cexp_d = nc.dram_tensor("cexp", [128, 1024], BF16, kind="ExternalInput")
    diagd_d = nc.dram_tensor("diagd", [128, 512], BF16, kind="ExternalInput")
    y_d = nc.dram_tensor("y", [TROWS, D], FP32, kind="ExternalOutput")

    with tile.TileContext(nc) as tc:
        with (
            tc.tile_pool(name="persist", bufs=1) as pp,
            tc.tile_pool(name="ptab", bufs=2) as ptab,
            tc.tile_pool(name="pu", bufs=2) as pu,
            tc.tile_pool(name="put", bufs=2) as put,
            tc.tile_pool(name="pbin", bufs=2) as pbin,
            tc.tile_pool(name="ptmp", bufs=2) as ptmp,
            tc.tile_pool(name="pg", bufs=2) as pg,
            tc.tile_pool(name="py3", bufs=3) as py3,
            tc.tile_pool(name="psT", bufs=2, space="PSUM") as psT,
            tc.tile_pool(name="psB", bufs=1, space="PSUM") as psB,
            tc.tile_pool(name="psC", bufs=2, space="PSUM") as psC,
        ):
            bcomb_s = pp.tile([128, 512], BF16, tag="bcomb")
            bcomb2_s = pp.tile([128, 512], BF16, tag="bcomb2")
            cexp_s = pp.tile([128, 1024], BF16, tag="cexp")
            diagd_s = pp.tile([128, 512], BF16, tag="diagd")
            rhoc_s = pp.tile([128, 1], FP32, tag="rhoc")
            ident = pp.tile([128, 128], BF16, tag="ident")
            zprev = pp.tile([128, BPC], FP32, tag="zprev")
            # params ride the sync HWDGE ring so the gpsimd SWDGE ring's
            # first work is the pass-0 u slabs (startup critical path).
            nc.sync.dma_start(out=bcomb_s, in_=bcomb_d[:, :])
            nc.sync.dma_start(out=bcomb2_s, in_=bcomb2_d[:, :])
            nc.sync.dma_start(out=cexp_s, in_=cexp_d[:, :])
            nc.sync.dma_start(out=diagd_s, in_=diagd_d[:, :])
            nc.sync.dma_start(out=rhoc_s, in_=rhoc_d[:, :])

            # per-pass state carried to the deferred P3
            state = [None] * Q
            ident_made = [False]

            def emit_p1_loads(q):
                """u slab DMAs (gpsimd/SWDGE, fp32->bf16 cast) + tables."""
                TQ = SIZES[q]
                NJ = TQ // 128
                u_tiles = []
                for b in range(BPC):
                    row0 = b * L + OFFS[q]
                    u_nat = pu.tile([128, NJ * 512], BF16,
                                    tag=f"u_nat{b}_{TQ}")
                    u_tiles.append(u_nat)
                    for hh in range(2):
                        rows = slice(row0 + hh * (TQ // 2),
                                     row0 + (hh + 1) * (TQ // 2))
                        srch = u_d[rows, :].rearrange("(j p) d -> p j d",
                                                      p=128)
                        seg = u_nat[:, hh * (NJ // 2) * 512:
                                    (hh + 1) * (NJ // 2) * 512]
                        nc.gpsimd.dma_start(
                            out=seg.rearrange("p (j d) -> p j d", j=NJ // 2),
                            in_=srch)
                if not ident_made[0]:
                    # emitted after the first u loads are queued on gpsimd
                    make_identity(nc, ident)
                    nc.gpsimd.memset(zprev, 0.0)
                    ident_made[0] = True
                cs = slice(OFFS[q], OFFS[q] + TQ)
                tcs = ptab.tile([128, TQ], BF16, tag=f"tcs{TQ}")
                tss = ptab.tile([128, TQ], BF16, tag=f"tss{TQ}")
                nc.sync.dma_start(out=tcs, in_=tcos_d[:, cs])
                nc.sync.dma_start(out=tss, in_=tsin_d[:, cs])
                return u_tiles, tcs, tss

            def make_p1_units(q, u_tiles):
                """Per-(b,g) transpose + uT evac + B-projection emitters."""
                TQ = SIZES[q]
                NG = TQ // 512                    # 512-t groups per batch
                uT_tiles = [None] * (BPC * NG)
                binb = pbin.tile([128, BPC * TQ], BF16, tag=f"binb{TQ}")
                binb2 = pbin.tile([128, BPC * TQ], BF16, tag=f"binb2{TQ}")

                def unit(b, g):
                    u_nat = u_tiles[b]
                    uT = put.tile([128, 2048], BF16, tag=f"uT{b}{g}")
                    uT_tiles[b * NG + g] = uT
                    for half in range(2):         # c-chunk pairs
                        pt = psT.tile([128, 1024], BF16, tag="pt")
                        for cc in range(2):
                            c = half * 2 + cc
                            for j2 in range(4):
                                col = (g * 4 + j2) * 512 + c * 128
                                nc.tensor.transpose(
                                    pt[:, cc * 512 + j2 * 128:
                                       cc * 512 + (j2 + 1) * 128],
                                    u_nat[:, col:col + 128], ident)
                        dst = uT[:, half * 1024:(half + 1) * 1024]
                        if half == 0:
                            nc.vector.tensor_copy(dst, pt)
                        else:
                            nc.scalar.copy(dst, pt)
                    pb = psB.tile([128, 512], FP32, tag="pb")
                    pb2 = psB.tile([128, 512], FP32, tag="pb2")
                    for c in range(4):
                        nc.tensor.matmul(
                            pb, bcomb_s[:, c * 128:(c + 1) * 128],
                            uT[:, c * 512:(c + 1) * 512],
                            start=(c == 0), stop=(c == 3))
                    for c in range(4):
                        nc.tensor.matmul(
                            pb2, bcomb2_s[:, c * 128:(c + 1) * 128],
                            uT[:, c * 512:(c + 1) * 512],
                            start=(c == 0), stop=(c == 3))
                    off = b * TQ + g * 512
                    nc.scalar.copy(binb[:, off:off + 512], pb)
                    nc.scalar.copy(binb2[:, off:off + 512], pb2)

                return binb, binb2, uT_tiles, unit

            def emit_p2(q, binb, binb2, tcs, tss):
                TQ = SIZES[q]
                # ---------------- P2: rotate + scan + inverse rotate -------
                rhob = rhoc_s.broadcast_to([128, TQ])
                g1, g2 = gtiles[q]
                sls = [slice(b * TQ, (b + 1) * TQ) for b in range(BPC)]
                tA = []
                for b in range(BPC):
                    tmpA = ptmp.tile([128, TQ], BF16, tag=f"tmpA{b}_{TQ}")
                    tmpB = ptmp.tile([128, TQ], BF16, tag=f"tmpB{b}_{TQ}")
                    tA.append(tmpA)
                    nc.vector.tensor_mul(tmpA, tcs, binb[:, sls[b]])
                    nc.vector.tensor_mul(tmpB, tss, binb2[:, sls[b]])
                    nc.vector.tensor_sub(binb[:, sls[b]], tmpA, tmpB)
                for b in range(BPC):
                    nc.vector.tensor_tensor_scan(
                        tA[b], rhob, binb[:, sls[b]],
                        zprev[:, b:b + 1], Alu.mult, Alu.add)
                    nc.vector.tensor_copy(zprev[:, b:b + 1],
                                          tA[b][:, TQ - 1:TQ])
                for b in range(BPC):
                    zt = tA[b]
                    g1b = g1[:, sls[b]]
                    g2b = g2[:, sls[b]]
                    nc.vector.tensor_mul(g1b[0:64, :], tcs[0:64, :],
                                         zt[0:64, :])
                    nc.vector.tensor_mul(g1b[64:128, :], tss[64:128, :],
                                         zt[64:128, :])
                    nc.vector.tensor_mul(g2b[0:64, :], tss[0:64, :],
                                         zt[0:64, :])
                    nc.vector.tensor_mul(g2b[64:128, :], tcs[64:128, :],
                                         zt[64:128, :])

            def p3_pair(q, b, pair):
                # ---------------- P3: C-projection + D*u + store -----------
                TQ = SIZES[q]
                NG = TQ // 512
                g1, g2 = gtiles[q]
                uT_tiles = state[q]
                row0 = b * L + OFFS[q]
                py = psC.tile([128, 1024], FP32, tag="py")
                for sub in range(2):
                    jj = pair * 2 + sub
                    g = jj // 4
                    j2 = jj % 4
                    off = b * TQ + jj * 128
                    uT = uT_tiles[b * NG + g]
                    ps_ = py[:, sub * 512:(sub + 1) * 512]
                    # full-region matmuls FIRST: start=True zeroes the whole
                    # region, so partial-region (diag) accumulate after.
                    nc.tensor.matmul(ps_, g1[:, off:off + 128],
                                     cexp_s[:, 0:512],
                                     start=True, stop=False,
                                     skip_group_check=True)
                    nc.tensor.matmul(ps_, g2[:, off:off + 128],
                                     cexp_s[:, 512:1024],
                                     start=False, stop=False,
                                     skip_group_check=True)
                    for c in range(4):            # D*u diagonal blocks
                        nc.tensor.matmul(
                            ps_[:, c * 128:(c + 1) * 128],
                            uT[:, c * 512 + j2 * 128:
                               c * 512 + (j2 + 1) * 128],
                            diagd_s[:, c * 128:(c + 1) * 128],
                            start=False, stop=(c == 3),
                            skip_group_check=True)
                ysl = py3.tile([128, 1024], FP32, tag="ysl")
                nc.scalar.copy(ysl, py)
                rows = slice(row0 + pair * 256, row0 + (pair + 1) * 256)
                dst = y_d[rows, :].rearrange("(j p) d -> p j d", p=128)
                nc.sync.dma_start(out=dst,
                                  in_=ysl.rearrange("p (j d) -> p j d", j=2))

            def p3_all(q):
                TQ = SIZES[q]
                for b in range(BPC):
                    for pair in range(TQ // 256):
                        p3_pair(q, b, pair)

            # v3 emission order (hardware-validated best): all of pass q's
            # P1 units, then its P2 scan block, then pass q-1's deferred
            # C-projection + stores.
            gtiles = [None] * Q
            for q in range(Q):
                u_tiles, tcs, tss = emit_p1_loads(q)
                binb, binb2, uT_tiles, unit = make_p1_units(q, u_tiles)
                state[q] = uT_tiles
                TQ = SIZES[q]
                g1t = pg.tile([128, BPC * TQ], BF16, tag=f"g1_{TQ}")
                g2t = pg.tile([128, BPC * TQ], BF16, tag=f"g2_{TQ}")
                gtiles[q] = (g1t, g2t)
                for b in range(BPC):
                    for g in range(TQ // 512):
                        unit(b, g)
                emit_p2(q, binb, binb2, tcs, tss)
                if q > 0:
                    p3_all(q - 1)
            p3_all(Q - 1)

    if split_waits:
        _split_multi_waits(nc, mybir)
    return nc


def kernel(**inputs):
    from concourse.bass_utils import run_bass_kernel_spmd

    u = np.ascontiguousarray(inputs["u"], dtype=np.float32)
    L = u.shape[1]
    params = _host_precompute(
        inputs["log_neg_real"], inputs["imag"], inputs["P_real"],
        inputs["P_imag"], inputs["Q_real"], inputs["Q_imag"],
        inputs["B_real"], inputs["B_imag"], inputs["C_real"],
        inputs["C_imag"], inputs["log_dt"], inputs["D"], L)

    if L not in _PROG_CACHE:
        _PROG_CACHE[L] = _build_program(L)
    nc = _PROG_CACHE[L]

    in_maps = []
    for c in range(NCORES):
        shard = np.ascontiguousarray(
            u[c * BPC:(c + 1) * BPC].reshape(BPC * L, u.shape[2]))
        m = {"u": shard}
        m.update(params)
        in_maps.append(m)

    kwargs = {}
    if TRACE:
        kwargs = dict(trace=True, stitch_traces=False)
    res = run_bass_kernel_spmd(nc, in_maps, core_ids=list(range(NCORES)),
                               **kwargs)
    global LAST_RESULTS
    LAST_RESULTS = res
    y = np.empty_like(u)
    for c in range(NCORES):
        y[c * BPC:(c + 1) * BPC] = res.results[c]["y"].reshape(BPC, L, u.shape[2])
    return y
